# revision 47
# baseline (speedup 1.0000x reference)
"""Trainium2 Bass kernel for nn_AttnBlock (GroupNorm + dense spatial attention).

Reference math (B=2, H=W=C=96, GROUPS=32, fp32):
    hn = GroupNorm32 over dim1(H) of x[B,H,W,C]  (stats over (3,W,C) per group)
    q/k/v = hn @ W* + b*
    scores = (q @ k^T) / sqrt(C)   over HW=9216 positions per batch
    o = softmax(scores) @ v
    out = x + o @ Wp + bp

Sharding (8 cores): core = (b, qc), b = core//4, qc = core%4. Each core holds
the full batch-b key-side tensors plus its 2304-query-row chunk and computes
attention for those rows. Output is the UNNORMALIZED projected tensor
pT = Wp^T @ (sum_n w v) plus the softmax denominator row r; the host computes
x + pT/r + bp (the division commutes with the linear Wp).

fp8 dataflow (empirically validated: logits z in [-2.8, 2.8], exp(z) <= 16.1
fits e4m3 max 240 with 2x margin; full-pipeline numpy sim rel err ~1e-3 vs
2e-2 tolerance; HW run of the v1 variant measured 1.13e-3):
  x8raw = e4m3(raw x), host-packed in DoubleRow split layout [49, 2, n]
        (channel c -> ki=c%49 is c - 49*ko, ko = c//49).
  x8s = x8raw * s[n] (GroupNorm scale row broadcast, multiplied on the Pool
        engine, which is otherwise idle), aug lanes (47,1)=shift[n] (fp8,
        DMA'd from stats rows per checkpoint), (48,1)=1.0 (one-time DMA).
        This is the classic scaled layout, so every downstream scale is a
        compile-time constant.
  A   = score strips: DoubleRow fp8 matmul, lhsT = x8s key-tile [49,2,128],
        rhs = q28 [49,2,mw]; exp arg = psum * 1/sqrt(C).
  exp = split across ACT (table Exp) and DVE (one-instruction Schraudolph:
        int8(round(z*SCALE*8/ln2 + 55.625)) bit-cast as e4m3; HW f32->int is
        round-to-nearest, probe-verified), interleaved A,D,A,D,...,A,A per 16.
        Both write fp8 pair tiles [128, 2, mw] (tile 2p -> ko 0, 2p+1 -> 1).
  C   = oT[97, mw] += vaug-pair^T @ exp-pair, DoubleRow fp8; vaug col 96 is
        the exact-ones column -> softmax denominator for free.

Engine discipline: only ACT/DVE can read PSUM, so the 162x1024-col exp
stream is the hard floor; everything movable is kept off them (x*s scaling
on Pool, squares for stats on DVE 2x-mode bf16, residual/bias/normalize on
host, DMAs only on the SP/Pool queues -- a dma_start's transfer time blocks
the issuing engine's queue).  PSUM: strips 3x[128,1024] (6 banks) + oT/pT
alternating (2 banks).
"""

import numpy as np
import ml_dtypes

B, H, W, C = 2, 96, 96, 96
GROUPS = 32
EPS = 1e-5
HW = H * W                 # 9216
NCORES = 8
QCH = HW // 4              # 2304 query rows per core
GSPAN = HW // GROUPS       # 288 rows per group
QGROUPS = QCH // GSPAN     # 8 groups per query chunk
SCALE = float(C) ** -0.5
CA = C + 2                 # aug channels: 96=shift, 97=ones
KI = CA // 2               # 49: DoubleRow contraction partitions
VA = C + 1                 # vaug cols: 96 = v, col 96 = ones
VPAD = 112                 # vaug per-tile stride (16-byte aligned)
NTILES = HW // 128         # 72 key tiles
PAIRS = NTILES // 2
CHK = 1152                 # stats/scale chunk: 4 whole groups
A8 = 8.0 / np.log(2.0)     # Schraudolph slope for e4m3
K8 = 55.625                # Schraudolph offset (RNE hardware rounding)
MBLOCKS = [(0, 1024), (1024, 1024), (2048, 256)]
ACT_NUM, ACT_DEN = 19, 32   # exp tiles on ACT : total (Bresenham-interleaved)

_compiled = {}


def _build_bass():
    import concourse.bass as bass
    import concourse.mybir as mybir
    import concourse.tile as tile

    # --- workaround: TRN2 allows one embedded sem-wait per instruction, but
    # TileContext piles every outstanding DMA-queue wait onto one tail drain.
    import bass_rust

    def _split_drain_and_barrier(self, tick_clock, wait_clock):
        nc = self.nc
        drain_inst = nc.sync.drain()
        wait_clock.add_sem_waits(
            drain_inst.ins, bass_rust.ScopedClock({None: tick_clock.global_clock})
        )
        si = drain_inst.ins.sync_info
        waits = list(si.on_wait) if si is not None and si.on_wait else []
        if len(waits) > 1:
            si.on_wait = waits[:1]
            for w in waits[1:]:
                extra = nc.sync.drain()
                esi = extra.ins.sync_info
                if esi is None:
                    extra.ins.sync_info = bass_rust.SyncInfo(on_wait=[w], on_update=[])
                else:
                    esi.on_wait = [w]
        nc.all_engine_barrier()
        assert self.sems is not None
        popped = nc._tile_sem_poison_stack.pop()
        assert popped is self._sem_poison
        nc.clear_and_free_semaphores(list(self.sems.allocated().values()))
        nc.all_engine_barrier()

    tile.TileContext._drain_and_barrier = _split_drain_and_barrier

    def _split_multiwaits(nc):
        """TRN2 ISA allows one embedded sem-wait per instruction; Tile's
        sem-assignment sometimes attaches several. Hoist extras onto
        engine-NOPs spliced immediately before the instruction."""
        n_split = 0
        for f in nc.m.functions:
            for bb in f.blocks:
                out = []
                changed = False
                for inst in bb.instructions:
                    si = getattr(inst, "sync_info", None)
                    if si is not None and si.on_wait and len(si.on_wait) > 1:
                        waits = list(si.on_wait)
                        for w in waits[:-1]:
                            n_split += 1
                            nop = bass_rust.InstNoOp(
                                name=f"WSPLIT-{n_split}", ins=[], outs=[]
                            )
                            nop.engine = inst.engine
                            nop.sync_info = bass_rust.SyncInfo(
                                on_wait=[w], on_update=[]
                            )
                            nc.register_instruction(nop)
                            out.append(nop)
                        si.on_wait = waits[-1:]
                        changed = True
                    out.append(inst)
                if changed:
                    bb.instructions = out
        return n_split

    f32 = mybir.dt.float32
    bf16 = mybir.dt.bfloat16
    fp8 = mybir.dt.float8e4
    i8 = mybir.dt.int8
    i32 = mybir.dt.int32
    AF = mybir.ActivationFunctionType
    ALU = mybir.AluOpType
    AX = mybir.AxisListType
    DR = mybir.MatmulPerfMode.DoubleRow

    nc = bass.Bass()

    x8d = nc.dram_tensor("x8d", [KI, 2 * HW], fp8, kind="ExternalInput")
    x8qd = nc.dram_tensor("x8qd", [KI, 2 * QCH], fp8, kind="ExternalInput")
    xb16 = nc.dram_tensor("xb16", [C, HW], bf16, kind="ExternalInput")
    xq16 = nc.dram_tensor("xq16", [C, QCH], bf16, kind="ExternalInput")
    wkqd = nc.dram_tensor("wkqd", [KI, 2 * VPAD], fp8, kind="ExternalInput")
    wv8d = nc.dram_tensor("wv8d", [KI, 2 * VPAD], fp8, kind="ExternalInput")
    wpd = nc.dram_tensor("wpd", [C, C], bf16, kind="ExternalInput")
    mask32d = nc.dram_tensor("mask32d", [C, GROUPS * GROUPS], bf16,
                             kind="ExternalInput")
    mask8d = nc.dram_tensor("mask8d", [C, QGROUPS * QGROUPS], bf16,
                            kind="ExternalInput")
    gRow = nc.dram_tensor("gRow", [GROUPS, GSPAN], f32, kind="ExternalInput")
    bRow = nc.dram_tensor("bRow", [GROUPS, GSPAN], f32, kind="ExternalInput")
    gRowQ = nc.dram_tensor("gRowQ", [QGROUPS, GSPAN], f32, kind="ExternalInput")
    bRowQ = nc.dram_tensor("bRowQ", [QGROUPS, GSPAN], f32, kind="ExternalInput")
    outP = nc.dram_tensor("outP", [C, QCH], f32, kind="ExternalOutput")
    outR = nc.dram_tensor("outR", [1, QCH], bf16, kind="ExternalOutput")
    # internal DRAM bounces, one per checkpoint (DRAM dep tracking is
    # whole-tensor; separate tensors keep later reads from serializing)
    sRowD = [nc.dram_tensor(f"sRowD{j}", [HW], bf16) for j in range(2)]
    scRowQD = nc.dram_tensor("scRowQD", [QCH], bf16)

    # checkpoint j: groups CKG[j][0]:CKG[j][1] (chunks {0,1} and {2..7})
    CKG = [(0, 8), (8, 32)]

    with tile.TileContext(nc) as tc:
        import contextlib

        with contextlib.ExitStack() as ctx:
            consts = ctx.enter_context(tc.tile_pool(name="consts", bufs=1))
            big = ctx.enter_context(tc.tile_pool(name="big", bufs=1))
            sps = ctx.enter_context(tc.tile_pool(name="sps", bufs=3, space="PSUM"))
            ops = ctx.enter_context(tc.tile_pool(name="ops", bufs=1, space="PSUM"))
            sqp = ctx.enter_context(tc.tile_pool(name="sq_sb", bufs=2))
            stb = ctx.enter_context(tc.tile_pool(name="stat_sb", bufs=2))
            scb = ctx.enter_context(tc.tile_pool(name="scb_sb", bufs=2))
            esb = ctx.enter_context(tc.tile_pool(name="exp_sb", bufs=16))
            osb = ctx.enter_context(tc.tile_pool(name="post_sb", bufs=2))

            # ---- big SBUF tensors ----
            x8r = big.tile([KI, 2 * HW], fp8)    # raw fp8 x
            x8 = big.tile([KI, 2 * HW], fp8)     # scaled + aug lanes
            xlr = big.tile([KI, 2 * QCH], fp8)
            sQrow = big.tile([KI, QCH], bf16)
            xb = big.tile([C, HW], bf16)
            xq = big.tile([C, QCH], bf16)
            q28 = big.tile([KI, 2 * QCH], fp8)
            vaug = big.tile([128, NTILES * VPAD], fp8)

            x83 = x8.rearrange("p (two n) -> p two n", two=2)
            xlr3 = xlr.rearrange("p (two n) -> p two n", two=2)
            q283 = q28.rearrange("p (two n) -> p two n", two=2)

            # ---- constants first: masks gate the stats matmuls and are tiny
            wkq_t = consts.tile([KI, 2 * VPAD], fp8)
            wv8_t = consts.tile([KI, 2 * VPAD], fp8)
            wp_t = consts.tile([C, C], bf16)
            m32_t = consts.tile([C, GROUPS * GROUPS], bf16)
            m8_t = consts.tile([C, QGROUPS * QGROUPS], bf16)
            for dst, src in [
                (m8_t, mask8d), (m32_t, mask32d), (wkq_t, wkqd),
                (wv8_t, wv8d),
            ]:
                nc.gpsimd.dma_start(out=dst, in_=src[:, :])

            grow = {}
            for key, gsrc, bsrc, ng in [
                ("L", gRowQ, bRowQ, QGROUPS), ("B", gRow, bRow, GROUPS)
            ]:
                gt = consts.tile([ng, GSPAN], f32, name=f"grow_{key}")
                nc.sync.dma_start(out=gt, in_=gsrc[:, :])
                bt = consts.tile([ng, GSPAN], f32, name=f"brow_{key}")
                nc.sync.dma_start(out=bt, in_=bsrc[:, :])
                grow[key] = (gt, bt)

            # ---- input loads (SP + Pool queues only; query side first:
            # xq gates the whole local-stats -> q28 critical chain) ----
            nc.sync.dma_start(out=xq, in_=xq16[:, :])
            nc.sync.dma_start(out=xlr, in_=x8qd[:, :])
            nc.sync.dma_start(out=wp_t, in_=wpd[:, :])
            for i in range(8):
                sl = slice(i * CHK, (i + 1) * CHK)
                (nc.sync if i % 2 == 0 else nc.gpsimd).dma_start(
                    out=xb[:, sl], in_=xb16[:, sl])
            for i in [0, 1, 2, 3]:
                sl2 = slice(2 * i * CHK, 2 * (i + 1) * CHK)
                nc.gpsimd.dma_start(out=x8r[:, sl2], in_=x8d[:, sl2])
            stats_acc = {
                "L": consts.tile([QGROUPS, 2], f32, name="accL"),
            }
            nc.vector.memset(stats_acc["L"], 0.0)

            # The KEY side's ones lane would only add a per-query constant to
            # every logit -- softmax-invariant -- so x8 lane 48/ko1 stays
            # zero and vaug's ones column is written by a strided memset.
            nc.vector.memset(
                vaug.rearrange("p (t v) -> p t v", v=VPAD)[:, :, C: C + 1], 1.0
            )

            CNT = 1.0 / (GSPAN * C)

            # batch stats accumulate across all 8 chunks into ONE persistent
            # psum tile (ops pool, released before mb0's oT): masked rows get
            # +0 from other chunks, so checkpoint reduces of row prefixes are
            # exact as soon as the covering chunks ran.
            bstat = ops.tile([GROUPS, 1024], f32, tag="op", name="bstat")

            def stats_chunk(x16, key, i):
                """Column sums of groups 4i..4i+3 of chunk i (lane aligned
                into group rows via one-hot masks)."""
                masks, ng = (m8_t, QGROUPS) if key == "L" else (m32_t, GROUPS)
                chunk = x16[:, i * CHK: (i + 1) * CHK]
                sq = sqp.tile([C, CHK], bf16, tag="sq", name="sq")
                nc.vector.tensor_mul(sq, chunk, chunk)
                # batch chunks form one accumulation group per checkpoint
                # phase ({0,1},{2,3},{4..7}); each checkpoint reads only its
                # own group rows, so the re-start zeroing is harmless.
                ts_ = bstat if key == "B" else sps.tile(
                    [128, 1024], f32, tag="sp", name="ts")
                first = (key == "L") or i in (0, 2)
                last = (key == "L") or i in (1, 7)
                ps_s = ts_[0:ng, 0:GSPAN]
                ps_q = ts_[0:ng, 512: 512 + GSPAN]
                for j in range(4):
                    g = 4 * i + j
                    sspan = slice(j * GSPAN, (j + 1) * GSPAN)
                    mk = masks[:, g * ng: (g + 1) * ng]
                    nc.tensor.matmul(
                        ps_s, mk, chunk[:, sspan],
                        start=(first and j == 0), stop=(last and j == 3)
                    )
                    nc.tensor.matmul(
                        ps_q, mk, sq[:, sspan],
                        start=(first and j == 0), stop=(last and j == 3)
                    )
                if key == "L":
                    red = stb.tile([GROUPS, 2], f32, tag="red", name="red")[:ng]
                    both = ts_[0:ng, :].rearrange(
                        "p (a s) -> p a s", a=2)[:, :, 0:GSPAN]
                    nc.vector.tensor_reduce(red, both, axis=AX.X, op=ALU.add)
                    nc.vector.tensor_add(stats_acc["L"], stats_acc["L"], red)

            def finish_side(key, g0=0, g1=None, on_sc32=None):
                """Per-group scalar math, computed on the base-0 prefix 0:g1
                (DVE requires base-0 partition starts; rows below g0 hold
                zeroed-group garbage -- finite, and never published): rsqrt
                via Quake seed + 2 Newton steps, then scale row sc32 (f32)
                and fp8 shift row sh8.  Callers publish only [g0:g1]."""
                if g1 is None:
                    g1 = QGROUPS if key == "L" else GROUPS
                sg = slice(0, g1)
                g_t, b_t = grow[key]
                g_t, b_t = g_t[sg], b_t[sg]
                if key == "L":
                    acc = stats_acc["L"][sg]
                else:
                    acc = stb.tile([GROUPS, 2], f32, tag="red", name="racc")[sg]
                    both = bstat[sg, :].rearrange(
                        "p (a s) -> p a s", a=2)[:, :, 0:GSPAN]
                    nc.vector.tensor_reduce(acc, both, axis=AX.X, op=ALU.add)
                st = stb.tile([GROUPS, 12], f32, tag="st", name="st")[sg]
                mex = st[:, 0:2]
                mean = st[:, 0:1]
                msq, var = st[:, 2:3], st[:, 3:4]
                veps, ti = st[:, 4:5], st[:, 5:6]
                ya, yb = st[:, 6:7], st[:, 7:8]
                rstd = st[:, 8:9]
                nc.vector.tensor_scalar_mul(mex, in0=acc[:, 0:2], scalar1=CNT)
                nc.vector.tensor_mul(msq, mean, mean)
                nc.vector.tensor_sub(var, st[:, 1:2], msq)
                nc.vector.tensor_scalar_add(veps, in0=var, scalar1=EPS)
                nc.vector.tensor_scalar(
                    out=ti.bitcast(i32), in0=veps.bitcast(i32),
                    scalar1=1, scalar2=-1, op0=ALU.arith_shift_right,
                    op1=ALU.bitwise_xor,
                )
                nc.vector.tensor_scalar_add(
                    rstd.bitcast(i32), in0=ti.bitcast(i32), scalar1=0x5F3759E0
                )
                for _ in range(2):
                    nc.vector.tensor_mul(ya, rstd, rstd)
                    nc.vector.tensor_mul(yb, ya, veps)
                    nc.vector.tensor_scalar(
                        out=yb, in0=yb, scalar1=-0.5, scalar2=1.5,
                        op0=ALU.mult, op1=ALU.add,
                    )
                    nc.vector.tensor_mul(rstd, rstd, yb)
                sc32 = stb.tile([GROUPS, GSPAN], f32, tag="sc", name="sc32")[sg]
                nc.vector.tensor_scalar_mul(sc32, in0=g_t, scalar1=rstd)
                if on_sc32 is not None:
                    on_sc32(sc32)
                ms32 = stb.tile([GROUPS, GSPAN], f32, tag="ms", name="ms32")[sg]
                nc.vector.tensor_scalar_mul(ms32, in0=sc32, scalar1=mean)
                sh8 = stb.tile([GROUPS, GSPAN], fp8, tag="sh", name="sh8")[sg]
                nc.vector.tensor_sub(sh8, b_t, ms32)
                return sc32, sh8

            # ---- local (query-side) chain ----
            for i in range(2):
                stats_chunk(xq, "L", i)
            def _pubL(sc):
                nc.gpsimd.dma_start(
                    out=scRowQD[0:QCH].rearrange("(g s) -> g s", s=GSPAN), in_=sc
                )
                nc.sync.dma_start(
                    out=sQrow,
                    in_=bass.AP(tensor=scRowQD, offset=0,
                                ap=[[0, KI], [1, QCH]]),
                )

            scL, shL = finish_side("L", on_sc32=_pubL)
            # query side stays RAW fp8: aug lanes get (shift/s, 1/s) and the
            # s[m] GroupNorm scale is applied at the q28 evac as a broadcast
            # row multiply (q28 = s[m] * Wkq @ xlr_aug).
            recL = stb.tile([QGROUPS, GSPAN], f32, tag="rc", name="recL")
            nc.vector.reciprocal(recL, scL)
            a0f = stb.tile([QGROUPS, GSPAN], f32, tag="a0", name="a0f")
            nc.vector.tensor_mul(a0f, shL, recL)
            a08 = stb.tile([QGROUPS, GSPAN], fp8, tag="a08", name="a08")
            nc.vector.tensor_copy(a08, a0f)
            a18 = stb.tile([QGROUPS, GSPAN], fp8, tag="a18", name="a18")
            nc.vector.tensor_copy(a18, recL)
            for lane, src in [(47, a08), (48, a18)]:
                nc.sync.dma_start(
                    out=xlr[lane: lane + 1, QCH: 2 * QCH].rearrange(
                        "p (g s) -> p g s", g=QGROUPS),
                    in_=src.rearrange("g (a s) -> g a s", a=1),
                )
            for i in [4, 5, 6, 7]:
                sl2 = slice(2 * i * CHK, 2 * (i + 1) * CHK)
                nc.sync.dma_start(out=x8r[:, sl2], in_=x8d[:, sl2])

            # ---- query chain: q28 = (WkAug @ WqAug^T) @ xl_aug directly,
            # one DoubleRow matmul per output ko-half, plain fp8 evacs ----
            q28_done = 0
            wkq3 = wkq_t.rearrange("p (two m) -> p two m", two=2)

            def emit_q28(upto):
                nonlocal q28_done
                while q28_done < upto:
                    w = min(512, upto - q28_done)
                    sl = slice(q28_done, q28_done + w)
                    t2 = sps.tile([128, 1024], f32, tag="sp", name="t2")
                    for ko in range(2):
                        nc.tensor.matmul(
                            t2[0:KI, ko * 512: ko * 512 + w],
                            wkq3[:, :, ko * KI: (ko + 1) * KI],
                            xlr3[:, :, sl], start=True, stop=True, perf_mode=DR,
                        )
                        nc.vector.tensor_tensor(
                            out=q283[:, ko, sl],
                            in0=t2[0:KI, ko * 512: ko * 512 + w],
                            in1=sQrow[:, sl], op=ALU.mult,
                        )
                    q28_done += w

            # ---- batch side: stats chunks, checkpoint finishes, x*s scale
            # (Pool), vaug production ----
            shB_of = {}

            def finish_ckpt(j):
                g0, g1 = CKG[j]

                def _pubB(sc):
                    nc.gpsimd.dma_start(
                        out=sRowD[j][g0 * GSPAN: g1 * GSPAN].rearrange(
                            "(g s) -> g s", s=GSPAN),
                        in_=sc[g0:g1],
                    )

                scB, shB = finish_side("B", g0, g1, on_sc32=_pubB)
                shB_of[j] = shB

            def scale_chunk(i, eng=None):
                """x8s chunk = x8raw * s row (Pool), ko1 over all 49
                partitions (aug lanes become initialized zeros), then the
                fp8 shift lane segment is DMA'd over lane 47/ko1."""
                j = 0 if i < 2 else 1
                mul = eng or nc.gpsimd
                dma = nc.gpsimd if mul is nc.gpsimd else nc.sync
                sl = slice(i * CHK, (i + 1) * CHK)
                sc_t = scb.tile([KI, CHK], bf16, tag="scb", name="scb")
                dma.dma_start(
                    out=sc_t,
                    in_=bass.AP(tensor=sRowD[j], offset=i * CHK,
                                ap=[[0, KI], [1, CHK]]),
                )
                x8r3 = x8r.rearrange("p (two n) -> p two n", two=2)
                mul.tensor_mul(x83[:, 0, sl], x8r3[:, 0, sl], sc_t)
                mul.tensor_mul(x83[:, 1, sl], x8r3[:, 1, sl], sc_t)
                nc.gpsimd.dma_start(
                    out=x8[47:48, HW + i * CHK: HW + (i + 1) * CHK].rearrange(
                        "p (g s) -> p g s", g=4),
                    in_=shB_of[j][4 * i: 4 * i + 4].rearrange(
                        "g (a s) -> g a s", a=1),
                )

            vaug_evac_alt = [0]

            def emit_vaug(i):
                """9 key tiles t = 9i..9i+8: DoubleRow matmul from scaled x8,
                plain fp8 pack evacs (5-tile + 4-tile)."""
                t0 = 9 * i
                tv = sps.tile([128, 1024], f32, tag="sp", name="tv")
                for jj in range(9):
                    off = jj * C if jj < 5 else 512 + (jj - 5) * C
                    nc.tensor.matmul(
                        tv[:, off: off + C],
                        x83[:, :, (t0 + jj) * 128: (t0 + jj + 1) * 128],
                        wv8_t.rearrange("p (two m) -> p two m", two=2)[:, :, 0:C],
                        start=True, stop=True, perf_mode=DR,
                    )
                for base, cnt in [(0, 5), (5, 4)]:
                    off = 0 if base == 0 else 512
                    src = tv[:, off: off + cnt * C].rearrange(
                        "p (c v) -> p c v", c=cnt)
                    dst = vaug[:, (t0 + base) * VPAD: (t0 + base + cnt) * VPAD
                               ].rearrange("p (c v) -> p c v", c=cnt)[:, :, 0:C]
                    if vaug_evac_alt[0] % 2 == 0:
                        nc.scalar.activation(dst, src, AF.Copy)
                    else:
                        nc.vector.tensor_copy(dst, src)
                    vaug_evac_alt[0] += 1

            stats_chunk(xb, "B", 0)
            emit_q28(512)
            stats_chunk(xb, "B", 1)
            finish_ckpt(0)
            scale_chunk(0)
            scale_chunk(1)
            emit_vaug(0)
            emit_q28(1024)
            emit_vaug(1)
            emit_q28(QCH)
            # vaug 2..7 and the last scale chunks are emitted inside the
            # attention stream (just ahead of demand) so their evac ops don't
            # head-of-line-block the ACT/DVE FIFOs before the first exp.

            # ---- attention m-blocks ----
            exp_idx = [0]

            def mb_open(mw):
                return {
                    "oT": ops.tile([VA, 1024], f32, tag="op", name="oT"),
                    "pend": [], "next": 0, "mw": mw,
                    "halves": [(h, min(512, mw - h)) for h in range(0, mw, 512)],
                }

            def _exp(dst, src, t):
                if (exp_idx[0] * ACT_NUM) % ACT_DEN < ACT_NUM:
                    nc.scalar.activation(dst, src, AF.Exp, scale=SCALE)
                else:
                    nc.vector.tensor_scalar(
                        out=dst.bitcast(i8), in0=src,
                        scalar1=SCALE * A8, scalar2=K8,
                        op0=ALU.mult, op1=ALU.add,
                    )
                exp_idx[0] += 1

            def mb_emit(st, mo, upto_pair):
                """mw=1024: one strip + one [128,1024] exp op per tile.
                mw=256: QUAD packing -- 4 tiles' A-outs share one strip, one
                exp op covers all 4; pend entries stay per-pair."""
                mw, halves = st["mw"], st["halves"]
                while st["next"] < upto_pair:
                    p = st["next"]
                    if mw == 256:
                        if p % 2 == 0:
                            exq = esb.tile([128, 1024], fp8, tag="ex", name="exq")
                            sp = sps.tile([128, 1024], f32, tag="sp", name="sp")
                            for j in range(4):
                                t = 2 * p + j
                                nc.tensor.matmul(
                                    sp[:, j * mw: (j + 1) * mw],
                                    x83[:, :, t * 128: (t + 1) * 128],
                                    q283[:, :, mo: mo + mw],
                                    start=True, stop=True, perf_mode=DR,
                                )
                            _exp(exq, sp, 2 * p)
                            st["quad"] = exq
                        ex = st["quad"].rearrange(
                            "q (four m) -> q four m", four=4
                        )[:, 2 * (p % 2): 2 * (p % 2) + 2, :]
                    else:
                        exf = esb.tile([128, 2 * mw], fp8, tag="ex", name="ex")
                        for ko in range(2):
                            t = 2 * p + ko
                            sp = sps.tile([128, 1024], f32, tag="sp", name="sp")
                            for h, hw_ in halves:
                                nc.tensor.matmul(
                                    sp[:, h: h + hw_],
                                    x83[:, :, t * 128: (t + 1) * 128],
                                    q283[:, :, mo + h: mo + h + hw_],
                                    start=True, stop=True, perf_mode=DR,
                                )
                            _exp(exf[:, ko * mw: (ko + 1) * mw], sp[:, 0:mw], t)
                        ex = exf.rearrange("q (two m) -> q two m", two=2)
                    st["pend"].append((p, ex))
                    st["next"] += 1
                    if len(st["pend"]) > 1:
                        _mb_c(st, mo)

            def _mb_c(st, mo):
                halves = st["halves"]
                p, ex3 = st["pend"].pop(0)
                va3 = vaug[:, 2 * p * VPAD: (2 * p + 2) * VPAD].rearrange(
                    "q (two m) -> q two m", two=2
                )[:, :, 0:VA]
                for h, hw_ in halves:
                    nc.tensor.matmul(
                        st["oT"][:, h: h + hw_],
                        va3, ex3[:, :, h: h + hw_],
                        start=(p == 0), stop=(p == PAIRS - 1), perf_mode=DR,
                    )

            def mb_finish(st, mo):
                while st["pend"]:
                    _mb_c(st, mo)
                mw = st["mw"]
                oTsb = osb.tile([VA, 1024], bf16, tag="oTsb", name="oTsb")
                nc.vector.tensor_copy(oTsb[:, 0:mw], st["oT"][:, 0:mw])
                nc.sync.dma_start(
                    out=outR[:, mo: mo + mw], in_=oTsb[C: C + 1, 0:mw]
                )
                pT = ops.tile([C, 1024], f32, tag="op", name="pT")
                for h, hw_ in st["halves"]:
                    nc.tensor.matmul(
                        pT[:, h: h + hw_], wp_t, oTsb[0:C, h: h + hw_],
                        start=True, stop=True,
                    )
                psb = osb.tile([C, 1024], f32, tag="psb", name="psb")
                nc.scalar.activation(psb[:, 0:mw], pT[:, 0:mw], AF.Copy)
                nc.sync.dma_start(out=outP[:, mo: mo + mw], in_=psb[:, 0:mw])

            # bridge m-block boundaries: pre-emit the next block's first pairs
            # before draining the previous block's tail so ACT/DVE never idle.
            st0 = mb_open(MBLOCKS[0][1])
            mb_emit(st0, MBLOCKS[0][0], 2)
            for i in range(2, 8):
                stats_chunk(xb, "B", i)
            finish_ckpt(1)
            scale_chunk(2)
            scale_chunk(3)
            scale_chunk(4)
            scale_chunk(5)
            mb_emit(st0, MBLOCKS[0][0], 6)
            emit_vaug(2)
            mb_emit(st0, MBLOCKS[0][0], 11)
            emit_vaug(3)
            mb_emit(st0, MBLOCKS[0][0], 16)
            emit_vaug(4)
            mb_emit(st0, MBLOCKS[0][0], 21)
            emit_vaug(5)
            mb_emit(st0, MBLOCKS[0][0], 25)
            scale_chunk(6)
            emit_vaug(6)
            mb_emit(st0, MBLOCKS[0][0], 29)
            scale_chunk(7)
            emit_vaug(7)
            mb_emit(st0, MBLOCKS[0][0], PAIRS)
            st1 = mb_open(MBLOCKS[1][1])
            mb_emit(st1, MBLOCKS[1][0], 3)
            mb_finish(st0, MBLOCKS[0][0])
            mb_emit(st1, MBLOCKS[1][0], PAIRS)
            st2 = mb_open(MBLOCKS[2][1])
            mb_emit(st2, MBLOCKS[2][0], 3)
            mb_finish(st1, MBLOCKS[1][0])
            mb_emit(st2, MBLOCKS[2][0], PAIRS)
            mb_finish(st2, MBLOCKS[2][0])

    _split_multiwaits(nc)
    return nc


def _prep_inputs(x, gamma, beta, Wq, bq, Wk, bk, Wv, bv, Wp, bp):
    bf16 = ml_dtypes.bfloat16
    e4 = ml_dtypes.float8_e4m3
    f32 = np.float32

    x2 = np.ascontiguousarray(np.asarray(x, f32).reshape(B, HW, C))
    gRow = np.repeat(np.asarray(gamma, f32), W).reshape(GROUPS, GSPAN)
    bRow = np.repeat(np.asarray(beta, f32), W).reshape(GROUPS, GSPAN)

    def split49(rows):
        """[98, n] -> [49, 2, n] with c = ko*49 + ki."""
        return np.stack([rows[0:KI], rows[KI:CA]], axis=1)

    Wqf, Wkf, Wvf, Wpf = (np.asarray(w, f32) for w in (Wq, Wk, Wv, Wp))
    WqAug = np.vstack([Wqf, Wqf.sum(0)[None, :], np.asarray(bq, f32)[None, :]])
    WkAug = np.vstack([Wkf, Wkf.sum(0)[None, :], np.asarray(bk, f32)[None, :]])
    # fold the q and k projections: scores = hn_aug^T (WkAug WqAug^T) hn_aug
    Wkq = (WkAug.astype(np.float64) @ WqAug.astype(np.float64).T).astype(f32)
    wkq = np.zeros((KI, 2, VPAD), e4)
    wkq[:, :, 0:CA] = split49(np.ascontiguousarray(Wkq.T).astype(e4))
    wkq8 = np.ascontiguousarray(wkq).reshape(KI, 2 * VPAD)
    WvAug = np.zeros((CA, VPAD), f32)
    WvAug[:C, :C] = Wvf
    WvAug[C, :C] = Wvf.sum(axis=0)
    WvAug[C + 1, :C] = np.asarray(bv, f32)
    WvAug[C + 1, C] = 1.0
    wv8 = np.ascontiguousarray(split49(WvAug.astype(e4)).reshape(KI, 2 * VPAD))

    mask32 = np.zeros((C, GROUPS * GROUPS), bf16)
    for g in range(GROUPS):
        mask32[:, g * GROUPS + g] = 1.0
    mask8 = np.zeros((C, QGROUPS * QGROUPS), bf16)
    for g in range(QGROUPS):
        mask8[:, g * QGROUPS + g] = 1.0

    in_maps = []
    for core in range(NCORES):
        b, qc = divmod(core, 4)
        xbT = np.ascontiguousarray(x2[b].T)          # [C, HW]
        x8aug = np.zeros((CA, HW), f32)
        x8aug[0:C] = xbT
        x8s = split49(x8aug.astype(e4))              # [49, 2, HW]
        qsl = slice(qc * QCH, (qc + 1) * QCH)
        in_maps.append({
            "x8d": np.ascontiguousarray(x8s).reshape(KI, 2 * HW),
            "x8qd": np.ascontiguousarray(x8s[:, :, qsl]).reshape(KI, 2 * QCH),
            "xb16": xbT.astype(bf16),
            "xq16": np.ascontiguousarray(xbT[:, qsl]).astype(bf16),
            "wkqd": wkq8, "wv8d": wv8,
            "wpd": Wpf.astype(bf16),
            "mask32d": mask32, "mask8d": mask8,
            "gRow": gRow, "bRow": bRow,
            "gRowQ": np.ascontiguousarray(gRow.reshape(4, QGROUPS, GSPAN)[qc]),
            "bRowQ": np.ascontiguousarray(bRow.reshape(4, QGROUPS, GSPAN)[qc]),
        })
    return in_maps


def _get_sharded_fn():
    """Build the 8-core shard_map callable once so repeated calls reuse the
    compiled NEFF executable."""
    if "fn" in _compiled:
        return _compiled["fn"]

    import jax
    from jax.sharding import Mesh, PartitionSpec
    from jax.experimental.shard_map import shard_map
    import concourse.mybir as mybir
    from concourse.bass2jax import (
        _bass_exec_p, install_neuronx_cc_hook, partition_id_tensor
    )

    if "nc" not in _compiled:
        _compiled["nc"] = _build_bass()
    nc = _compiled["nc"]
    install_neuronx_cc_hook()

    pname = nc.partition_id_tensor.name if nc.partition_id_tensor else None
    in_names, out_names, out_avals = [], [], []
    for alloc in nc.m.functions[0].allocations:
        if not isinstance(alloc, mybir.MemoryLocationSet):
            continue
        name = alloc.memorylocations[0].name
        if alloc.kind == "ExternalInput":
            if name != pname:
                in_names.append(name)
        elif alloc.kind == "ExternalOutput":
            out_names.append(name)
            out_avals.append(
                jax.core.ShapedArray(
                    tuple(alloc.tensor_shape), mybir.dt.np(alloc.dtype)
                )
            )
    n_params = len(in_names)
    all_names = in_names + out_names
    if pname is not None:
        all_names = all_names + [pname]

    def _body(*args):
        operands = list(args)
        if pname is not None:
            operands.append(partition_id_tensor())
        outs = _bass_exec_p.bind(
            *operands,
            out_avals=tuple(out_avals),
            in_names=tuple(all_names),
            out_names=tuple(out_names),
            lowering_input_output_aliases=(),
            sim_require_finite=True,
            sim_require_nnan=True,
            nc=nc,
        )
        return tuple(outs)

    devices = jax.devices()[:NCORES]
    mesh = Mesh(np.asarray(devices), ("core",))
    sharded = jax.jit(
        shard_map(
            _body, mesh=mesh,
            in_specs=(PartitionSpec("core"),) * (n_params + len(out_names)),
            out_specs=(PartitionSpec("core"),) * len(out_names),
            check_rep=False,
        ),
        keep_unused=True,
    )

    from jax.sharding import NamedSharding

    shard = NamedSharding(mesh, PartitionSpec("core"))

    def put(in_maps):
        dev = [
            jax.device_put(
                np.concatenate(
                    [np.asarray(in_maps[c][nm]) for c in range(NCORES)], axis=0
                ),
                shard,
            )
            for nm in in_names
        ]
        dev += [
            jax.device_put(
                np.zeros((NCORES * a.shape[0], *a.shape[1:]), a.dtype), shard
            )
            for a in out_avals
        ]
        return dev

    def execute(dev_in):
        return sharded(*dev_in)

    def run(in_maps):
        out_arrs = execute(put(in_maps))
        return {
            nm: np.asarray(out_arrs[i]).reshape(NCORES, *out_avals[i].shape)
            for i, nm in enumerate(out_names)
        }

    _compiled["fn"] = (run, out_names, put, execute)
    _compiled["mkchain"] = (sharded, in_names, out_names, _body)
    return _compiled["fn"]


def kernel(x, gamma, beta, Wq, bq, Wk, bk, Wv, bv, Wp, bp):
    run = _get_sharded_fn()[0]
    in_maps = _prep_inputs(
        np.asarray(x, np.float32), gamma, beta, Wq, bq, Wk, bk, Wv, bv, Wp, bp
    )
    res = run(in_maps)
    pT = res["outP"].astype(np.float64)    # [8, C, QCH]
    r = res["outR"].astype(np.float64)     # [8, 1, QCH]

    x2 = np.asarray(x, np.float64).reshape(B, HW, C)
    # the key-side ones lane is dropped on device, so the v-bias is folded in
    # here: o_true = o_dev + bv  =>  out = x + o_dev Wp + (bv Wp + bp)
    bpf = (np.asarray(bp, np.float64)
           + np.asarray(bv, np.float64) @ np.asarray(Wp, np.float64))
    out = np.empty((B, HW, C), np.float32)
    for core in range(NCORES):
        b, qc = divmod(core, 4)
        sl = slice(qc * QCH, (qc + 1) * QCH)
        out[b, sl, :] = (
            x2[b, sl, :] + (pT[core] / r[core]).T + bpf[None, :]
        ).astype(np.float32)
    return out.reshape(B, H, W, C)


# revision 52
# speedup vs baseline: 1.9128x; 1.9128x over previous
"""Trainium2 Bass kernel for nn_AttnBlock (GroupNorm + dense spatial attention).

Reference math (B=2, H=W=C=96, GROUPS=32, fp32):
    hn = GroupNorm32 over dim1(H) of x[B,H,W,C]  (stats over (3,W,C) per group)
    q/k/v = hn @ W* + b*
    scores = (q @ k^T) / sqrt(C)   over HW=9216 positions per batch
    o = softmax(scores) @ v
    out = x + o @ Wp + bp

Sharding (8 cores): core = (b, qc), b = core//4, qc = core%4. Each core holds
the full batch-b key-side tensors plus its 2304-query-row chunk and computes
attention for those rows. Output is the UNNORMALIZED projected tensor
pT = Wp^T @ (sum_n w v) plus the softmax denominator row r; the host computes
x + pT/r + bp (the division commutes with the linear Wp).

fp8 dataflow (empirically validated: logits z in [-2.8, 2.8], exp(z) <= 16.1
fits e4m3 max 240 with 2x margin; full-pipeline numpy sim rel err ~1e-3 vs
2e-2 tolerance; HW run of the v1 variant measured 1.13e-3):
  x8raw = e4m3(raw x), host-packed in DoubleRow split layout [49, 2, n]
        (channel c -> ki=c%49 is c - 49*ko, ko = c//49).
  x8s = x8raw * s[n] (GroupNorm scale row broadcast, multiplied on the Pool
        engine, which is otherwise idle), aug lane (47,1)=shift[n] (fp8,
        DMA'd from stats rows per checkpoint); lane (48,1) stays zero -- the
        key-side ones lane would only add a per-query constant to each logit
        (softmax-invariant), so it is dropped, vaug's ones column is written
        by a strided memset, and bv folds into the host-side bias.  The
        query side stays RAW: its aug lanes are (shift/s, 1/s) and s[m] is
        applied at the q28 evac as a broadcast-row multiply.  Every
        downstream scale is then a compile-time constant.
  A   = score strips: DoubleRow fp8 matmul, lhsT = x8s key-tile [49,2,128],
        rhs = q28 [49,2,mw]; exp arg = psum * 1/sqrt(C).
  exp = split across ACT (table Exp) and DVE (one-instruction Schraudolph:
        int8(round(z*SCALE*8/ln2 + 55.625)) bit-cast as e4m3; HW f32->int is
        round-to-nearest, probe-verified), interleaved A,D,A,D,...,A,A per 16.
        Both write fp8 pair tiles [128, 2, mw] (tile 2p -> ko 0, 2p+1 -> 1).
  C   = oT[97, mw] += vaug-pair^T @ exp-pair, DoubleRow fp8; vaug col 96 is
        the exact-ones column -> softmax denominator for free.

Engine discipline: only ACT/DVE can read PSUM, so the 162x1024-col exp
stream is the hard floor; everything movable is kept off them (x*s scaling
on Pool, squares for stats on DVE 2x-mode bf16, residual/bias/normalize on
host, DMAs only on the SP/Pool queues -- a dma_start's transfer time blocks
the issuing engine's queue).  PSUM: strips 3x[128,1024] (6 banks) + oT/pT
alternating (2 banks).
"""

import numpy as np
import ml_dtypes

B, H, W, C = 2, 96, 96, 96
GROUPS = 32
EPS = 1e-5
HW = H * W                 # 9216
NCORES = 8
QCH = HW // 4              # 2304 query rows per core
GSPAN = HW // GROUPS       # 288 rows per group
QGROUPS = QCH // GSPAN     # 8 groups per query chunk
SCALE = float(C) ** -0.5
CA = C + 2                 # aug channels: 96=shift, 97=ones
KI = CA // 2               # 49: DoubleRow contraction partitions
VA = C + 1                 # vaug cols: 96 = v, col 96 = ones
VPAD = 112                 # vaug per-tile stride (16-byte aligned)
NTILES = HW // 128         # 72 key tiles
PAIRS = NTILES // 2
CHK = 1152                 # stats/scale chunk: 4 whole groups
A8 = 8.0 / np.log(2.0)     # Schraudolph slope for e4m3
K8 = 55.625                # Schraudolph offset (RNE hardware rounding)
MBLOCKS = [(0, 1024), (1024, 1024), (2048, 256)]
ACT_NUM, ACT_DEN = 19, 32   # exp tiles on ACT : total (Bresenham-interleaved)

_compiled = {}


def _build_bass():
    import concourse.bass as bass
    import concourse.mybir as mybir
    import concourse.tile as tile

    # --- workaround: TRN2 allows one embedded sem-wait per instruction, but
    # TileContext piles every outstanding DMA-queue wait onto one tail drain.
    import bass_rust

    def _split_drain_and_barrier(self, tick_clock, wait_clock):
        nc = self.nc
        drain_inst = nc.sync.drain()
        wait_clock.add_sem_waits(
            drain_inst.ins, bass_rust.ScopedClock({None: tick_clock.global_clock})
        )
        si = drain_inst.ins.sync_info
        waits = list(si.on_wait) if si is not None and si.on_wait else []
        if len(waits) > 1:
            si.on_wait = waits[:1]
            for w in waits[1:]:
                extra = nc.sync.drain()
                esi = extra.ins.sync_info
                if esi is None:
                    extra.ins.sync_info = bass_rust.SyncInfo(on_wait=[w], on_update=[])
                else:
                    esi.on_wait = [w]
        nc.all_engine_barrier()
        assert self.sems is not None
        popped = nc._tile_sem_poison_stack.pop()
        assert popped is self._sem_poison
        nc.clear_and_free_semaphores(list(self.sems.allocated().values()))
        nc.all_engine_barrier()

    tile.TileContext._drain_and_barrier = _split_drain_and_barrier

    def _split_multiwaits(nc):
        """TRN2 ISA allows one embedded sem-wait per instruction; Tile's
        sem-assignment sometimes attaches several. Hoist extras onto
        engine-NOPs spliced immediately before the instruction."""
        n_split = 0
        for f in nc.m.functions:
            for bb in f.blocks:
                out = []
                changed = False
                for inst in bb.instructions:
                    si = getattr(inst, "sync_info", None)
                    if si is not None and si.on_wait and len(si.on_wait) > 1:
                        waits = list(si.on_wait)
                        for w in waits[:-1]:
                            n_split += 1
                            nop = bass_rust.InstNoOp(
                                name=f"WSPLIT-{n_split}", ins=[], outs=[]
                            )
                            nop.engine = inst.engine
                            nop.sync_info = bass_rust.SyncInfo(
                                on_wait=[w], on_update=[]
                            )
                            nc.register_instruction(nop)
                            out.append(nop)
                        si.on_wait = waits[-1:]
                        changed = True
                    out.append(inst)
                if changed:
                    bb.instructions = out
        return n_split

    f32 = mybir.dt.float32
    bf16 = mybir.dt.bfloat16
    fp8 = mybir.dt.float8e4
    i8 = mybir.dt.int8
    i32 = mybir.dt.int32
    AF = mybir.ActivationFunctionType
    ALU = mybir.AluOpType
    AX = mybir.AxisListType
    DR = mybir.MatmulPerfMode.DoubleRow

    nc = bass.Bass()

    x8d = nc.dram_tensor("x8d", [KI, 2 * HW], fp8, kind="ExternalInput")
    x8qd = nc.dram_tensor("x8qd", [KI, 2 * QCH], fp8, kind="ExternalInput")
    xb16 = nc.dram_tensor("xb16", [C, HW], bf16, kind="ExternalInput")
    xq16 = nc.dram_tensor("xq16", [C, QCH], bf16, kind="ExternalInput")
    wkqd = nc.dram_tensor("wkqd", [KI, 2 * VPAD], fp8, kind="ExternalInput")
    wv8d = nc.dram_tensor("wv8d", [KI, 2 * VPAD], fp8, kind="ExternalInput")
    wpd = nc.dram_tensor("wpd", [C, C], bf16, kind="ExternalInput")
    mask32d = nc.dram_tensor("mask32d", [C, GROUPS * GROUPS], bf16,
                             kind="ExternalInput")
    mask8d = nc.dram_tensor("mask8d", [C, QGROUPS * QGROUPS], bf16,
                            kind="ExternalInput")
    gRow = nc.dram_tensor("gRow", [GROUPS, GSPAN], f32, kind="ExternalInput")
    bRow = nc.dram_tensor("bRow", [GROUPS, GSPAN], f32, kind="ExternalInput")
    gRowQ = nc.dram_tensor("gRowQ", [QGROUPS, GSPAN], f32, kind="ExternalInput")
    bRowQ = nc.dram_tensor("bRowQ", [QGROUPS, GSPAN], f32, kind="ExternalInput")
    outP = nc.dram_tensor("outP", [C, QCH], f32, kind="ExternalOutput")
    outR = nc.dram_tensor("outR", [1, QCH], bf16, kind="ExternalOutput")
    # internal DRAM bounces, one per checkpoint (DRAM dep tracking is
    # whole-tensor; separate tensors keep later reads from serializing)
    sRowD = [nc.dram_tensor(f"sRowD{j}", [HW], bf16) for j in range(2)]
    scRowQD = nc.dram_tensor("scRowQD", [QCH], bf16)

    # checkpoint j: groups CKG[j][0]:CKG[j][1] (chunks {0,1} and {2..7})
    CKG = [(0, 8), (8, 32)]

    with tile.TileContext(nc) as tc:
        import contextlib

        with contextlib.ExitStack() as ctx:
            consts = ctx.enter_context(tc.tile_pool(name="consts", bufs=1))
            big = ctx.enter_context(tc.tile_pool(name="big", bufs=1))
            sps = ctx.enter_context(tc.tile_pool(name="sps", bufs=3, space="PSUM"))
            ops = ctx.enter_context(tc.tile_pool(name="ops", bufs=1, space="PSUM"))
            sqp = ctx.enter_context(tc.tile_pool(name="sq_sb", bufs=2))
            stb = ctx.enter_context(tc.tile_pool(name="stat_sb", bufs=2))
            scb = ctx.enter_context(tc.tile_pool(name="scb_sb", bufs=2))
            esb = ctx.enter_context(tc.tile_pool(name="exp_sb", bufs=16))
            osb = ctx.enter_context(tc.tile_pool(name="post_sb", bufs=2))

            # ---- big SBUF tensors ----
            x8r = big.tile([KI, 2 * HW], fp8)    # raw fp8 x
            x8 = big.tile([KI, 2 * HW], fp8)     # scaled + aug lanes
            xlr = big.tile([KI, 2 * QCH], fp8)
            sQrow = big.tile([KI, QCH], bf16)
            xb = big.tile([C, HW], bf16)
            xq = big.tile([C, QCH], bf16)
            q28 = big.tile([KI, 2 * QCH], fp8)
            vaug = big.tile([128, NTILES * VPAD], fp8)

            x83 = x8.rearrange("p (two n) -> p two n", two=2)
            xlr3 = xlr.rearrange("p (two n) -> p two n", two=2)
            q283 = q28.rearrange("p (two n) -> p two n", two=2)

            # ---- constants first: masks gate the stats matmuls and are tiny
            wkq_t = consts.tile([KI, 2 * VPAD], fp8)
            wv8_t = consts.tile([KI, 2 * VPAD], fp8)
            wp_t = consts.tile([C, C], bf16)
            m32_t = consts.tile([C, GROUPS * GROUPS], bf16)
            m8_t = consts.tile([C, QGROUPS * QGROUPS], bf16)
            for dst, src in [
                (m8_t, mask8d), (m32_t, mask32d), (wkq_t, wkqd),
                (wv8_t, wv8d),
            ]:
                nc.gpsimd.dma_start(out=dst, in_=src[:, :])

            grow = {}
            for key, gsrc, bsrc, ng in [
                ("L", gRowQ, bRowQ, QGROUPS), ("B", gRow, bRow, GROUPS)
            ]:
                gt = consts.tile([ng, GSPAN], f32, name=f"grow_{key}")
                nc.sync.dma_start(out=gt, in_=gsrc[:, :])
                bt = consts.tile([ng, GSPAN], f32, name=f"brow_{key}")
                nc.sync.dma_start(out=bt, in_=bsrc[:, :])
                grow[key] = (gt, bt)

            # ---- input loads (SP + Pool queues only).  xq/xb0 gate the two
            # stats chains; x8r flat-halves 0 (Pool) and 4 (SP) gate the
            # first scale mul; everything else follows.
            nc.sync.dma_start(out=xq, in_=xq16[:, :])
            nc.sync.dma_start(out=xb[:, 0:CHK], in_=xb16[:, 0:CHK])
            nc.sync.dma_start(
                out=x8r[:, 8 * CHK: 10 * CHK], in_=x8d[:, 8 * CHK: 10 * CHK])
            nc.sync.dma_start(out=xlr, in_=x8qd[:, :])
            for i in [2, 4, 6]:
                sl = slice(i * CHK, (i + 1) * CHK)
                nc.sync.dma_start(out=xb[:, sl], in_=xb16[:, sl])
            for i in [1, 3, 5, 7]:
                sl = slice(i * CHK, (i + 1) * CHK)
                nc.gpsimd.dma_start(out=xb[:, sl], in_=xb16[:, sl])
            for i in [0, 1, 2, 3]:
                sl2 = slice(2 * i * CHK, 2 * (i + 1) * CHK)
                nc.gpsimd.dma_start(out=x8r[:, sl2], in_=x8d[:, sl2])
            nc.sync.dma_start(out=wp_t, in_=wpd[:, :])
            stats_acc = {
                "L": consts.tile([QGROUPS, 2], f32, name="accL"),
            }
            nc.vector.memset(stats_acc["L"], 0.0)

            # The KEY side's ones lane would only add a per-query constant to
            # every logit -- softmax-invariant -- so x8 lane 48/ko1 stays
            # zero and vaug's ones column is written by a strided memset.
            nc.vector.memset(
                vaug.rearrange("p (t v) -> p t v", v=VPAD)[:, :, C: C + 1], 1.0
            )

            CNT = 1.0 / (GSPAN * C)

            # batch stats accumulate across all 8 chunks into ONE persistent
            # psum tile (ops pool, released before mb0's oT): masked rows get
            # +0 from other chunks, so checkpoint reduces of row prefixes are
            # exact as soon as the covering chunks ran.
            bstat = ops.tile([GROUPS, 1024], f32, tag="op", name="bstat")

            def stats_chunk(x16, key, i):
                """Column sums of groups 4i..4i+3 of chunk i (lane aligned
                into group rows via one-hot masks)."""
                masks, ng = (m8_t, QGROUPS) if key == "L" else (m32_t, GROUPS)
                chunk = x16[:, i * CHK: (i + 1) * CHK]
                sq = sqp.tile([C, CHK], bf16, tag="sq", name="sq")
                nc.vector.tensor_mul(sq, chunk, chunk)
                # batch chunks form one accumulation group per checkpoint
                # phase ({0,1},{2,3},{4..7}); each checkpoint reads only its
                # own group rows, so the re-start zeroing is harmless.
                ts_ = bstat if key == "B" else sps.tile(
                    [128, 1024], f32, tag="sp", name="ts")
                first = (key == "L") or i in (0, 2)
                last = (key == "L") or i in (1, 7)
                ps_s = ts_[0:ng, 0:GSPAN]
                ps_q = ts_[0:ng, 512: 512 + GSPAN]
                for j in range(4):
                    g = 4 * i + j
                    sspan = slice(j * GSPAN, (j + 1) * GSPAN)
                    mk = masks[:, g * ng: (g + 1) * ng]
                    nc.tensor.matmul(
                        ps_s, mk, chunk[:, sspan],
                        start=(first and j == 0), stop=(last and j == 3)
                    )
                    nc.tensor.matmul(
                        ps_q, mk, sq[:, sspan],
                        start=(first and j == 0), stop=(last and j == 3)
                    )
                if key == "L":
                    red = stb.tile([GROUPS, 2], f32, tag="red", name="red")[:ng]
                    both = ts_[0:ng, :].rearrange(
                        "p (a s) -> p a s", a=2)[:, :, 0:GSPAN]
                    nc.vector.tensor_reduce(red, both, axis=AX.X, op=ALU.add)
                    nc.vector.tensor_add(stats_acc["L"], stats_acc["L"], red)

            def finish_side(key, g0=0, g1=None, on_sc32=None, eng=None):
                """Per-group scalar math, computed on the base-0 prefix 0:g1
                (DVE requires base-0 partition starts; rows below g0 hold
                zeroed-group garbage -- finite, and never published): rsqrt
                via Quake seed + 2 Newton steps, then scale row sc32 (f32)
                and fp8 shift row sh8.  Callers publish only [g0:g1]."""
                if g1 is None:
                    g1 = QGROUPS if key == "L" else GROUPS
                sg = slice(0, g1)
                g_t, b_t = grow[key]
                g_t, b_t = g_t[sg], b_t[sg]
                if key == "L":
                    acc = stats_acc["L"][sg]
                else:
                    acc = stb.tile([GROUPS, 2], f32, tag="red", name="racc")[sg]
                    both = bstat[sg, :].rearrange(
                        "p (a s) -> p a s", a=2)[:, :, 0:GSPAN]
                    nc.vector.tensor_reduce(acc, both, axis=AX.X, op=ALU.add)
                ev = eng or nc.vector
                st = stb.tile([GROUPS, 12], f32, tag="st", name="st")[sg]
                mex = st[:, 0:2]
                mean = st[:, 0:1]
                msq, var = st[:, 2:3], st[:, 3:4]
                veps, ti = st[:, 4:5], st[:, 5:6]
                ya, yb = st[:, 6:7], st[:, 7:8]
                rstd = st[:, 8:9]
                ev.tensor_scalar_mul(mex, in0=acc[:, 0:2], scalar1=CNT)
                ev.tensor_mul(msq, mean, mean)
                ev.tensor_sub(var, st[:, 1:2], msq)
                ev.tensor_scalar_add(veps, in0=var, scalar1=EPS)
                ev.tensor_scalar(
                    out=ti.bitcast(i32), in0=veps.bitcast(i32),
                    scalar1=1, scalar2=-1, op0=ALU.arith_shift_right,
                    op1=ALU.bitwise_xor,
                )
                ev.tensor_scalar_add(
                    rstd.bitcast(i32), in0=ti.bitcast(i32), scalar1=0x5F3759E0
                )
                for _ in range(2):
                    ev.tensor_mul(ya, rstd, rstd)
                    ev.tensor_mul(yb, ya, veps)
                    ev.tensor_scalar(
                        out=yb, in0=yb, scalar1=-0.5, scalar2=1.5,
                        op0=ALU.mult, op1=ALU.add,
                    )
                    ev.tensor_mul(rstd, rstd, yb)
                sc32 = stb.tile([GROUPS, GSPAN], f32, tag="sc", name="sc32")[sg]
                ev.tensor_scalar_mul(sc32, in0=g_t, scalar1=rstd)
                if on_sc32 is not None:
                    on_sc32(sc32)
                ms32 = stb.tile([GROUPS, GSPAN], f32, tag="ms", name="ms32")[sg]
                ev.tensor_scalar_mul(ms32, in0=sc32, scalar1=mean)
                sh8 = stb.tile([GROUPS, GSPAN], fp8, tag="sh", name="sh8")[sg]
                ev.tensor_sub(sh8, b_t, ms32)
                return sc32, sh8

            # ---- stats: batch chunks 0,1 first (they gate the longer
            # Pool-side ckpt0 -> scale -> vaug chain), then the local chain
            for i in range(2):
                stats_chunk(xb, "B", i)
            for i in range(2):
                stats_chunk(xq, "L", i)
            def _pubL(sc):
                nc.gpsimd.dma_start(
                    out=scRowQD[0:QCH].rearrange("(g s) -> g s", s=GSPAN), in_=sc
                )
                nc.sync.dma_start(
                    out=sQrow,
                    in_=bass.AP(tensor=scRowQD, offset=0,
                                ap=[[0, KI], [1, QCH]]),
                )

            scL, shL = finish_side("L", on_sc32=_pubL)
            # query side stays RAW fp8: aug lanes get (shift/s, 1/s) and the
            # s[m] GroupNorm scale is applied at the q28 evac as a broadcast
            # row multiply (q28 = s[m] * Wkq @ xlr_aug).
            recL = stb.tile([QGROUPS, GSPAN], f32, tag="rc", name="recL")
            nc.vector.reciprocal(recL, scL)
            a0f = stb.tile([QGROUPS, GSPAN], f32, tag="a0", name="a0f")
            nc.vector.tensor_mul(a0f, shL, recL)
            a08 = stb.tile([QGROUPS, GSPAN], fp8, tag="a08", name="a08")
            nc.vector.tensor_copy(a08, a0f)
            a18 = stb.tile([QGROUPS, GSPAN], fp8, tag="a18", name="a18")
            nc.vector.tensor_copy(a18, recL)
            for lane, src in [(47, a08), (48, a18)]:
                nc.sync.dma_start(
                    out=xlr[lane: lane + 1, QCH: 2 * QCH].rearrange(
                        "p (g s) -> p g s", g=QGROUPS),
                    in_=src.rearrange("g (a s) -> g a s", a=1),
                )
            for i in [5, 6, 7]:
                sl2 = slice(2 * i * CHK, 2 * (i + 1) * CHK)
                nc.sync.dma_start(out=x8r[:, sl2], in_=x8d[:, sl2])

            # ---- query chain: q28 = (WkAug @ WqAug^T) @ xl_aug directly,
            # one DoubleRow matmul per output ko-half, plain fp8 evacs ----
            q28_done = 0
            wkq3 = wkq_t.rearrange("p (two m) -> p two m", two=2)

            def emit_q28(upto):
                nonlocal q28_done
                while q28_done < upto:
                    w = min(512, upto - q28_done)
                    sl = slice(q28_done, q28_done + w)
                    t2 = sps.tile([128, 1024], f32, tag="sp", name="t2")
                    for ko in range(2):
                        nc.tensor.matmul(
                            t2[0:KI, ko * 512: ko * 512 + w],
                            wkq3[:, :, ko * KI: (ko + 1) * KI],
                            xlr3[:, :, sl], start=True, stop=True, perf_mode=DR,
                        )
                        nc.vector.tensor_tensor(
                            out=q283[:, ko, sl],
                            in0=t2[0:KI, ko * 512: ko * 512 + w],
                            in1=sQrow[:, sl], op=ALU.mult,
                        )
                    q28_done += w

            # ---- batch side: stats chunks, checkpoint finishes, x*s scale
            # (Pool), vaug production ----
            shB_of = {}

            def finish_ckpt(j):
                g0, g1 = CKG[j]

                def _pubB(sc):
                    nc.gpsimd.dma_start(
                        out=sRowD[j][g0 * GSPAN: g1 * GSPAN].rearrange(
                            "(g s) -> g s", s=GSPAN),
                        in_=sc[g0:g1],
                    )

                scB, shB = finish_side("B", g0, g1, on_sc32=_pubB)
                shB_of[j] = shB

            def scale_chunk(i, eng=None):
                """x8s chunk = x8raw * s row (Pool), ko1 over all 49
                partitions (aug lanes become initialized zeros), then the
                fp8 shift lane segment is DMA'd over lane 47/ko1."""
                j = 0 if i < 2 else 1
                mul = eng or nc.gpsimd
                dma = nc.gpsimd if mul is nc.gpsimd else nc.sync
                sl = slice(i * CHK, (i + 1) * CHK)
                sc_t = scb.tile([KI, CHK], bf16, tag="scb", name="scb")
                dma.dma_start(
                    out=sc_t,
                    in_=bass.AP(tensor=sRowD[j], offset=i * CHK,
                                ap=[[0, KI], [1, CHK]]),
                )
                x8r3 = x8r.rearrange("p (two n) -> p two n", two=2)
                mul.tensor_mul(x83[:, 0, sl], x8r3[:, 0, sl], sc_t)
                mul.tensor_mul(x83[:, 1, sl], x8r3[:, 1, sl], sc_t)
                nc.gpsimd.dma_start(
                    out=x8[47:48, HW + i * CHK: HW + (i + 1) * CHK].rearrange(
                        "p (g s) -> p g s", g=4),
                    in_=shB_of[j][4 * i: 4 * i + 4].rearrange(
                        "g (a s) -> g a s", a=1),
                )

            vaug_evac_alt = [0]

            def emit_vaug(i):
                """9 key tiles t = 9i..9i+8: DoubleRow matmul from scaled x8,
                plain fp8 pack evacs (5-tile + 4-tile)."""
                t0 = 9 * i
                tv = sps.tile([128, 1024], f32, tag="sp", name="tv")
                for jj in range(9):
                    off = jj * C if jj < 5 else 512 + (jj - 5) * C
                    nc.tensor.matmul(
                        tv[:, off: off + C],
                        x83[:, :, (t0 + jj) * 128: (t0 + jj + 1) * 128],
                        wv8_t.rearrange("p (two m) -> p two m", two=2)[:, :, 0:C],
                        start=True, stop=True, perf_mode=DR,
                    )
                for base, cnt in [(0, 5), (5, 4)]:
                    off = 0 if base == 0 else 512
                    src = tv[:, off: off + cnt * C].rearrange(
                        "p (c v) -> p c v", c=cnt)
                    dst = vaug[:, (t0 + base) * VPAD: (t0 + base + cnt) * VPAD
                               ].rearrange("p (c v) -> p c v", c=cnt)[:, :, 0:C]
                    if vaug_evac_alt[0] % 2 == 0:
                        nc.scalar.activation(dst, src, AF.Copy)
                    else:
                        nc.vector.tensor_copy(dst, src)
                    vaug_evac_alt[0] += 1

            emit_q28(512)
            finish_ckpt(0)
            scale_chunk(0)
            scale_chunk(1)
            emit_vaug(0)
            emit_q28(1024)
            emit_vaug(1)
            emit_q28(QCH)
            # vaug 2..7 and the last scale chunks are emitted inside the
            # attention stream (just ahead of demand) so their evac ops don't
            # head-of-line-block the ACT/DVE FIFOs before the first exp.

            # ---- attention m-blocks ----
            exp_idx = [0]

            def mb_open(mw):
                return {
                    "oT": ops.tile([VA, 1024], f32, tag="op", name="oT"),
                    "pend": [], "next": 0, "mw": mw,
                    "halves": [(h, min(512, mw - h)) for h in range(0, mw, 512)],
                }

            def _exp(dst, src, t):
                if (exp_idx[0] * ACT_NUM) % ACT_DEN < ACT_NUM:
                    nc.scalar.activation(dst, src, AF.Exp, scale=SCALE)
                else:
                    nc.vector.tensor_scalar(
                        out=dst.bitcast(i8), in0=src,
                        scalar1=SCALE * A8, scalar2=K8,
                        op0=ALU.mult, op1=ALU.add,
                    )
                exp_idx[0] += 1

            def mb_emit(st, mo, upto_pair):
                """mw=1024: one strip + one [128,1024] exp op per tile.
                mw=256: QUAD packing -- 4 tiles' A-outs share one strip, one
                exp op covers all 4; pend entries stay per-pair."""
                mw, halves = st["mw"], st["halves"]
                while st["next"] < upto_pair:
                    p = st["next"]
                    if mw == 256:
                        if p % 2 == 0:
                            exq = esb.tile([128, 1024], fp8, tag="ex", name="exq")
                            sp = sps.tile([128, 1024], f32, tag="sp", name="sp")
                            for j in range(4):
                                t = 2 * p + j
                                nc.tensor.matmul(
                                    sp[:, j * mw: (j + 1) * mw],
                                    x83[:, :, t * 128: (t + 1) * 128],
                                    q283[:, :, mo: mo + mw],
                                    start=True, stop=True, perf_mode=DR,
                                )
                            _exp(exq, sp, 2 * p)
                            st["quad"] = exq
                        ex = st["quad"].rearrange(
                            "q (four m) -> q four m", four=4
                        )[:, 2 * (p % 2): 2 * (p % 2) + 2, :]
                    else:
                        exf = esb.tile([128, 2 * mw], fp8, tag="ex", name="ex")
                        for ko in range(2):
                            t = 2 * p + ko
                            sp = sps.tile([128, 1024], f32, tag="sp", name="sp")
                            for h, hw_ in halves:
                                nc.tensor.matmul(
                                    sp[:, h: h + hw_],
                                    x83[:, :, t * 128: (t + 1) * 128],
                                    q283[:, :, mo + h: mo + h + hw_],
                                    start=True, stop=True, perf_mode=DR,
                                )
                            _exp(exf[:, ko * mw: (ko + 1) * mw], sp[:, 0:mw], t)
                        ex = exf.rearrange("q (two m) -> q two m", two=2)
                    st["pend"].append((p, ex))
                    st["next"] += 1
                    if len(st["pend"]) > 1:
                        _mb_c(st, mo)

            def _mb_c(st, mo):
                halves = st["halves"]
                p, ex3 = st["pend"].pop(0)
                va3 = vaug[:, 2 * p * VPAD: (2 * p + 2) * VPAD].rearrange(
                    "q (two m) -> q two m", two=2
                )[:, :, 0:VA]
                for h, hw_ in halves:
                    nc.tensor.matmul(
                        st["oT"][:, h: h + hw_],
                        va3, ex3[:, :, h: h + hw_],
                        start=(p == 0), stop=(p == PAIRS - 1), perf_mode=DR,
                    )

            def mb_finish(st, mo):
                while st["pend"]:
                    _mb_c(st, mo)
                mw = st["mw"]
                oTsb = osb.tile([VA, 1024], bf16, tag="oTsb", name="oTsb")
                nc.vector.tensor_copy(oTsb[:, 0:mw], st["oT"][:, 0:mw])
                nc.sync.dma_start(
                    out=outR[:, mo: mo + mw], in_=oTsb[C: C + 1, 0:mw]
                )
                pT = ops.tile([C, 1024], f32, tag="op", name="pT")
                for h, hw_ in st["halves"]:
                    nc.tensor.matmul(
                        pT[:, h: h + hw_], wp_t, oTsb[0:C, h: h + hw_],
                        start=True, stop=True,
                    )
                psb = osb.tile([C, 1024], f32, tag="psb", name="psb")
                nc.scalar.activation(psb[:, 0:mw], pT[:, 0:mw], AF.Copy)
                nc.sync.dma_start(out=outP[:, mo: mo + mw], in_=psb[:, 0:mw])

            # bridge m-block boundaries: pre-emit the next block's first pairs
            # before draining the previous block's tail so ACT/DVE never idle.
            st0 = mb_open(MBLOCKS[0][1])
            mb_emit(st0, MBLOCKS[0][0], 2)
            for i in range(2, 8):
                stats_chunk(xb, "B", i)
            finish_ckpt(1)
            scale_chunk(2)
            scale_chunk(3)
            scale_chunk(4)
            scale_chunk(5)
            mb_emit(st0, MBLOCKS[0][0], 6)
            emit_vaug(2)
            mb_emit(st0, MBLOCKS[0][0], 11)
            emit_vaug(3)
            mb_emit(st0, MBLOCKS[0][0], 16)
            emit_vaug(4)
            mb_emit(st0, MBLOCKS[0][0], 21)
            emit_vaug(5)
            mb_emit(st0, MBLOCKS[0][0], 25)
            scale_chunk(6)
            emit_vaug(6)
            mb_emit(st0, MBLOCKS[0][0], 29)
            scale_chunk(7)
            emit_vaug(7)
            mb_emit(st0, MBLOCKS[0][0], PAIRS)
            st1 = mb_open(MBLOCKS[1][1])
            mb_emit(st1, MBLOCKS[1][0], 3)
            mb_finish(st0, MBLOCKS[0][0])
            mb_emit(st1, MBLOCKS[1][0], PAIRS)
            st2 = mb_open(MBLOCKS[2][1])
            mb_emit(st2, MBLOCKS[2][0], 3)
            mb_finish(st1, MBLOCKS[1][0])
            mb_emit(st2, MBLOCKS[2][0], PAIRS)
            mb_finish(st2, MBLOCKS[2][0])

    _split_multiwaits(nc)
    return nc


def _prep_inputs(x, gamma, beta, Wq, bq, Wk, bk, Wv, bv, Wp, bp):
    bf16 = ml_dtypes.bfloat16
    e4 = ml_dtypes.float8_e4m3
    f32 = np.float32

    x2 = np.ascontiguousarray(np.asarray(x, f32).reshape(B, HW, C))
    gRow = np.repeat(np.asarray(gamma, f32), W).reshape(GROUPS, GSPAN)
    bRow = np.repeat(np.asarray(beta, f32), W).reshape(GROUPS, GSPAN)

    def split49(rows):
        """[98, n] -> [49, 2, n] with c = ko*49 + ki."""
        return np.stack([rows[0:KI], rows[KI:CA]], axis=1)

    Wqf, Wkf, Wvf, Wpf = (np.asarray(w, f32) for w in (Wq, Wk, Wv, Wp))
    WqAug = np.vstack([Wqf, Wqf.sum(0)[None, :], np.asarray(bq, f32)[None, :]])
    WkAug = np.vstack([Wkf, Wkf.sum(0)[None, :], np.asarray(bk, f32)[None, :]])
    # fold the q and k projections: scores = hn_aug^T (WkAug WqAug^T) hn_aug
    Wkq = (WkAug.astype(np.float64) @ WqAug.astype(np.float64).T).astype(f32)
    wkq = np.zeros((KI, 2, VPAD), e4)
    wkq[:, :, 0:CA] = split49(np.ascontiguousarray(Wkq.T).astype(e4))
    wkq8 = np.ascontiguousarray(wkq).reshape(KI, 2 * VPAD)
    WvAug = np.zeros((CA, VPAD), f32)
    WvAug[:C, :C] = Wvf
    WvAug[C, :C] = Wvf.sum(axis=0)
    WvAug[C + 1, :C] = np.asarray(bv, f32)
    WvAug[C + 1, C] = 1.0
    wv8 = np.ascontiguousarray(split49(WvAug.astype(e4)).reshape(KI, 2 * VPAD))

    mask32 = np.zeros((C, GROUPS * GROUPS), bf16)
    for g in range(GROUPS):
        mask32[:, g * GROUPS + g] = 1.0
    mask8 = np.zeros((C, QGROUPS * QGROUPS), bf16)
    for g in range(QGROUPS):
        mask8[:, g * QGROUPS + g] = 1.0

    in_maps = []
    for core in range(NCORES):
        b, qc = divmod(core, 4)
        xbT = np.ascontiguousarray(x2[b].T)          # [C, HW]
        x8aug = np.zeros((CA, HW), f32)
        x8aug[0:C] = xbT
        x8s = split49(x8aug.astype(e4))              # [49, 2, HW]
        qsl = slice(qc * QCH, (qc + 1) * QCH)
        in_maps.append({
            "x8d": np.ascontiguousarray(x8s).reshape(KI, 2 * HW),
            "x8qd": np.ascontiguousarray(x8s[:, :, qsl]).reshape(KI, 2 * QCH),
            "xb16": xbT.astype(bf16),
            "xq16": np.ascontiguousarray(xbT[:, qsl]).astype(bf16),
            "wkqd": wkq8, "wv8d": wv8,
            "wpd": Wpf.astype(bf16),
            "mask32d": mask32, "mask8d": mask8,
            "gRow": gRow, "bRow": bRow,
            "gRowQ": np.ascontiguousarray(gRow.reshape(4, QGROUPS, GSPAN)[qc]),
            "bRowQ": np.ascontiguousarray(bRow.reshape(4, QGROUPS, GSPAN)[qc]),
        })
    return in_maps


def _get_sharded_fn():
    """Build the 8-core shard_map callable once so repeated calls reuse the
    compiled NEFF executable."""
    if "fn" in _compiled:
        return _compiled["fn"]

    import jax
    from jax.sharding import Mesh, PartitionSpec
    from jax.experimental.shard_map import shard_map
    import concourse.mybir as mybir
    from concourse.bass2jax import (
        _bass_exec_p, install_neuronx_cc_hook, partition_id_tensor
    )

    if "nc" not in _compiled:
        _compiled["nc"] = _build_bass()
    nc = _compiled["nc"]
    install_neuronx_cc_hook()

    pname = nc.partition_id_tensor.name if nc.partition_id_tensor else None
    in_names, out_names, out_avals = [], [], []
    for alloc in nc.m.functions[0].allocations:
        if not isinstance(alloc, mybir.MemoryLocationSet):
            continue
        name = alloc.memorylocations[0].name
        if alloc.kind == "ExternalInput":
            if name != pname:
                in_names.append(name)
        elif alloc.kind == "ExternalOutput":
            out_names.append(name)
            out_avals.append(
                jax.core.ShapedArray(
                    tuple(alloc.tensor_shape), mybir.dt.np(alloc.dtype)
                )
            )
    n_params = len(in_names)
    all_names = in_names + out_names
    if pname is not None:
        all_names = all_names + [pname]

    def _body(*args):
        operands = list(args)
        if pname is not None:
            operands.append(partition_id_tensor())
        outs = _bass_exec_p.bind(
            *operands,
            out_avals=tuple(out_avals),
            in_names=tuple(all_names),
            out_names=tuple(out_names),
            lowering_input_output_aliases=(),
            sim_require_finite=True,
            sim_require_nnan=True,
            nc=nc,
        )
        return tuple(outs)

    devices = jax.devices()[:NCORES]
    mesh = Mesh(np.asarray(devices), ("core",))
    sharded = jax.jit(
        shard_map(
            _body, mesh=mesh,
            in_specs=(PartitionSpec("core"),) * (n_params + len(out_names)),
            out_specs=(PartitionSpec("core"),) * len(out_names),
            check_rep=False,
        ),
        keep_unused=True,
    )

    from jax.sharding import NamedSharding

    shard = NamedSharding(mesh, PartitionSpec("core"))

    def put(in_maps):
        dev = [
            jax.device_put(
                np.concatenate(
                    [np.asarray(in_maps[c][nm]) for c in range(NCORES)], axis=0
                ),
                shard,
            )
            for nm in in_names
        ]
        dev += [
            jax.device_put(
                np.zeros((NCORES * a.shape[0], *a.shape[1:]), a.dtype), shard
            )
            for a in out_avals
        ]
        return dev

    def execute(dev_in):
        return sharded(*dev_in)

    def run(in_maps):
        out_arrs = execute(put(in_maps))
        return {
            nm: np.asarray(out_arrs[i]).reshape(NCORES, *out_avals[i].shape)
            for i, nm in enumerate(out_names)
        }

    _compiled["fn"] = (run, out_names, put, execute)
    _compiled["mkchain"] = (sharded, in_names, out_names, _body)
    return _compiled["fn"]


def kernel(x, gamma, beta, Wq, bq, Wk, bk, Wv, bv, Wp, bp):
    run = _get_sharded_fn()[0]
    in_maps = _prep_inputs(
        np.asarray(x, np.float32), gamma, beta, Wq, bq, Wk, bk, Wv, bv, Wp, bp
    )
    res = run(in_maps)
    pT = res["outP"].astype(np.float64)    # [8, C, QCH]
    r = res["outR"].astype(np.float64)     # [8, 1, QCH]

    x2 = np.asarray(x, np.float64).reshape(B, HW, C)
    # the key-side ones lane is dropped on device, so the v-bias is folded in
    # here: o_true = o_dev + bv  =>  out = x + o_dev Wp + (bv Wp + bp)
    bpf = (np.asarray(bp, np.float64)
           + np.asarray(bv, np.float64) @ np.asarray(Wp, np.float64))
    out = np.empty((B, HW, C), np.float32)
    for core in range(NCORES):
        b, qc = divmod(core, 4)
        sl = slice(qc * QCH, (qc + 1) * QCH)
        out[b, sl, :] = (
            x2[b, sl, :] + (pT[core] / r[core]).T + bpf[None, :]
        ).astype(np.float32)
    return out.reshape(B, H, W, C)
